# revision 27
# baseline (speedup 1.0000x reference)
"""Distributed Trainium2 (Bass/Tile) kernel for the KPCL contrastive loss.

Math (matches the jax reference):
  x1 = f + sign(f) * normalize(n1, 1e-8) * 0.1
  x2 = x1 + sign(x1) * normalize(n2, 1e-8) * 0.1
     = sign(f) * (|f| + u1/max(10*||u1||,1e-7) + u2/max(10*||u2||,1e-7))
  p  = relu(x2 @ W1 + b1) @ W2 + b2
  z  = p / max(||p||, 1e-6)
  sim = z @ z_all.T / T ;  lse_i = log(sum_j exp(sim_ij)) ; pos_i = sim_ii
  loss = mean(-pos + lse) + log(2)

Sharding: rows (N=8192) split across 8 cores, 1024 rows each. Each core
computes its z block in transposed layout zT [128, 8, 128] (bf16), and
the zT columns are AllGathered in two 512-column chunks (each [128,512]
bf16, fired as soon as its half of the local rows is done so the
collective overlaps the rest of phase A). A tiny dummy AllGather is
issued first so the one-time collectives bootstrap barrier runs
concurrently with phase A instead of gating the real data transfers.
Phase C computes the row-block of sim = zT_m^T @ z_all^T as bf16
128x512 matmuls with fused exp+row-sum on the activation engine.
Per-core output is [128, 16] (per-partition log-sum-exp values and diag
terms); the host does the final scalar reduction.

Engine split in phase A: Act does squares/abs/sign/sqrt/relu, DVE does
the augment adds + norms, Pool (gpsimd) does the sign-multiply and all
PSUM->SBUF copies, PE does transposes + the (bf16) projection matmuls.
"""

import sys

for _p in ("/opt/trn_rl_repo",):
    if _p not in sys.path:
        sys.path.append(_p)

import numpy as np

import concourse.bass as bass
import concourse.tile as tile
from concourse import mybir
from concourse.bass_utils import run_bass_kernel_spmd
from concourse.masks import make_identity

F32 = mybir.dt.float32
BF16 = mybir.dt.bfloat16
U32 = mybir.dt.uint32

N_CORES = 8
N = 8192
ROWS = N // N_CORES          # 1024 rows per core
D_IN = 512
D_PROJ = 128
TEMP = 0.15
P = 128                      # partitions
NBLK = ROWS // P             # 8 row-blocks per core
GB = 4                       # blocks per group (AllGather chunk)
NGRP = NBLK // GB            # 2 groups
INV_T = 1.0 / TEMP

AF = mybir.ActivationFunctionType
OP = mybir.AluOpType


def split_excess_waits(nc: bass.Bass, max_waits: int = 1) -> int:
    """Hoist excess sem waits onto same-engine nop carriers.

    The walrus build in this image rejects instructions carrying more
    than ~2 sync commands ("Too many sync wait commands"), but Tile's
    wait assignment freely emits 2-3 waits per instruction. Splitting
    the waits onto preceding nop instructions on the same engine queue
    is semantically identical (engine program order is preserved).
    """
    nmoved = 0
    for f in nc.m.functions:
        for b in f.blocks:
            il = b.instructions
            i = 0
            while i < len(il):
                inst = il[i]
                si = inst.sync_info
                if si is None or not si.on_wait or len(si.on_wait) <= max_waits:
                    i += 1
                    continue
                eng = inst.engine
                if eng is None:
                    i += 1
                    continue
                waits = list(si.on_wait)
                keep = waits[-max_waits:]
                excess = waits[:-max_waits]
                carriers = []
                for w in excess:
                    nop = nc.engines[eng].nop().ins
                    for f2 in nc.m.functions:
                        for b2 in f2.blocks:
                            try:
                                b2.instructions.remove(nop)
                            except ValueError:
                                pass
                    nop.sync_info = mybir.SyncInfo(on_wait=[w], on_update=[])
                    carriers.append(nop)
                inst.sync_info = mybir.SyncInfo(on_wait=keep,
                                                on_update=list(si.on_update))
                for c in reversed(carriers):
                    il.insert(i, c)
                i += 1 + len(carriers)
                nmoved += len(excess)
    return nmoved


def build_nc() -> bass.Bass:
    nc = bass.Bass("TRN2", target_bir_lowering=False, debug=False,
                   num_devices=N_CORES)

    f_d = nc.dram_tensor("features", [ROWS, D_IN], F32, kind="ExternalInput")
    u1_d = nc.dram_tensor("noise1", [ROWS, D_IN], F32, kind="ExternalInput")
    u2_d = nc.dram_tensor("noise2", [ROWS, D_IN], F32, kind="ExternalInput")
    w1_d = nc.dram_tensor("W1", [D_IN, D_PROJ], F32, kind="ExternalInput")
    b1_d = nc.dram_tensor("b1", [D_PROJ, 1], F32, kind="ExternalInput")
    w2_d = nc.dram_tensor("W2", [D_PROJ, D_PROJ], F32, kind="ExternalInput")
    b2_d = nc.dram_tensor("b2", [D_PROJ, 1], F32, kind="ExternalInput")
    out_d = nc.dram_tensor("out", [P, 2 * NBLK], F32, kind="ExternalOutput")

    # dummy collective to pull the one-time bootstrap barrier early
    # (gathers a tiny zero scratch tensor; the result is unused)
    dmy_in = nc.dram_tensor("dmy_in", [P, 1], F32)
    dmy_out = nc.dram_tensor("dmy_out", [N_CORES * P, 1], F32,
                             addr_space="Shared")

    # collective bounce buffers per chunk (AG output must be Shared)
    zTb = [nc.dram_tensor(f"zTb{g}", [P, GB, P], BF16) for g in range(NGRP)]
    zallb = [nc.dram_tensor(f"zallb{g}", [N_CORES * P, GB * P], BF16,
                            addr_space="Shared") for g in range(NGRP)]

    with tile.TileContext(nc) as tc:
        with (
            tc.tile_pool(name="singles", bufs=1) as singles,
            tc.tile_pool(name="grp", bufs=2) as grp,
            tc.tile_pool(name="wk", bufs=2) as wk,
            tc.tile_pool(name="sj", bufs=2) as sj,
            tc.tile_pool(name="zr", bufs=2) as zr,
            tc.tile_pool(name="small", bufs=2) as small,
            tc.tile_pool(name="expsc", bufs=2) as expsc,
        ):
            # fire the dummy collective first: its completion is unused,
            # it only exists to absorb the bootstrap barrier early.
            zbias = singles.tile([P, 1], F32)
            nc.gpsimd.memset(zbias[:], 0.0)
            nc.sync.dma_start(out=dmy_in[:, :], in_=zbias[:])
            nc.gpsimd.collective_compute(
                "AllGather", OP.bypass, ins=[dmy_in[:, :]],
                outs=[dmy_out[:, :]],
                replica_groups=[list(range(N_CORES))],
            )

            # ---- constants / persistent tiles ----
            w1f = singles.tile([P, 4, P], F32)
            for c in range(4):
                nc.sync.dma_start(w1f[:, c, :], w1_d[c * P:(c + 1) * P, :])
            w2f = singles.tile([P, P], F32)
            nc.sync.dma_start(w2f[:], w2_d[:, :])
            b1t = singles.tile([P, 1], F32)
            nc.sync.dma_start(b1t[:], b1_d[:, :])
            b2t = singles.tile([P, 1], F32)
            nc.sync.dma_start(b2t[:], b2_d[:, :])
            w1b = singles.tile([P, 4, P], BF16)
            nc.vector.tensor_copy(w1b[:], w1f[:])
            w2b = singles.tile([P, P], BF16)
            nc.vector.tensor_copy(w2b[:], w2f[:])

            ident = singles.tile([P, P], F32)
            make_identity(nc, ident[:])
            identb = singles.tile([P, P], BF16)
            nc.vector.tensor_copy(identb[:], ident[:])

            s1all = singles.tile([P, NBLK], F32)    # ||u1||^2 per row
            s2all = singles.tile([P, NBLK], F32)
            nsqP = singles.tile([P, NBLK], F32)     # ||p||^2 per row (col layout)
            zT = singles.tile([P, NBLK, P], BF16)   # z^T for this core
            zallT = [singles.tile([P, N_CORES, GB * P], BF16,
                                  name=f"zallT{g}", tag=f"zallT{g}")
                     for g in range(NGRP)]
            sacc = singles.tile([P, NBLK, 4], F32)  # partial exp row-sums
            Stot = singles.tile([P, NBLK], F32)
            outb = singles.tile([P, 2 * NBLK], F32)  # [logS | pos]

            # ---- all input DMAs up front (block-major) ----
            ftg, u1g, u2g = [], [], []
            for g in range(NGRP):
                ftg.append(grp.tile([P, GB, D_IN], F32, name=f"ft{g}",
                                    tag="F"))
                u1g.append(grp.tile([P, GB, D_IN], F32, name=f"u1t{g}",
                                    tag="U1"))
                u2g.append(grp.tile([P, GB, D_IN], F32, name=f"u2t{g}",
                                    tag="U2"))
            for g in range(NGRP):
                for mm in range(GB):
                    rs = slice((g * GB + mm) * P, (g * GB + mm + 1) * P)
                    nc.sync.dma_start(u1g[g][:, mm, :], u1_d[rs, :])
                    nc.sync.dma_start(ftg[g][:, mm, :], f_d[rs, :])
                    nc.sync.dma_start(u2g[g][:, mm, :], u2_d[rs, :])

            # =========== Phase A: augment + projection + normalize ==========
            with (
                tc.tile_pool(name="psT", bufs=2, space="PSUM") as psT,
                tc.tile_pool(name="psH", bufs=2, space="PSUM") as psH,
                tc.tile_pool(name="psQ", bufs=2, space="PSUM") as psQ,
                tc.tile_pool(name="psZ", bufs=2, space="PSUM") as psZ,
            ):
                # --- stage 1 (all blocks): row sums of squares, sign(f) ---
                sgnt = []
                for m in range(NBLK):
                    g, mm = divmod(m, GB)
                    junk = sj.tile([P, D_IN], BF16, tag="sqj")
                    nc.vector.scalar_tensor_tensor(
                        out=junk[:], in0=u1g[g][:, mm, :], scalar=1.0,
                        in1=u1g[g][:, mm, :], op0=OP.mult, op1=OP.mult,
                        accum_out=s1all[:, m:m + 1])
                    junk = sj.tile([P, D_IN], BF16, tag="sqj")
                    nc.vector.scalar_tensor_tensor(
                        out=junk[:], in0=u2g[g][:, mm, :], scalar=1.0,
                        in1=u2g[g][:, mm, :], op0=OP.mult, op1=OP.mult,
                        accum_out=s2all[:, m:m + 1])
                    sg = wk.tile([P, D_IN], F32, tag="sgn", bufs=9)
                    nc.scalar.activation(sg[:], ftg[g][:, mm, :], AF.Sign,
                                         bias=zbias[:])
                    sgnt.append(sg)

                # --- stage 2 (batched): r = 1/max(10*||u||, 1e-7) ---
                n1gt = small.tile([P, NBLK], F32, tag="n1g")
                nc.scalar.activation(n1gt[:], s1all[:], AF.Sqrt,
                                     bias=zbias[:], scale=100.0)
                n2gt = small.tile([P, NBLK], F32, tag="n2g")
                nc.scalar.activation(n2gt[:], s2all[:], AF.Sqrt,
                                     bias=zbias[:], scale=100.0)
                n1c = small.tile([P, NBLK], F32, tag="n1c")
                nc.vector.tensor_scalar(out=n1c[:], in0=n1gt[:],
                                        scalar1=1e-7, scalar2=None,
                                        op0=OP.max)
                r1a = small.tile([P, NBLK], F32, tag="r1a")
                nc.vector.reciprocal(r1a[:], n1c[:])
                n2c = small.tile([P, NBLK], F32, tag="n2c")
                nc.vector.tensor_scalar(out=n2c[:], in0=n2gt[:],
                                        scalar1=1e-7, scalar2=None,
                                        op0=OP.max)
                r2a = small.tile([P, NBLK], F32, tag="r2a")
                nc.vector.reciprocal(r2a[:], n2c[:])

                for g in range(NGRP):
                    g4 = slice(g * GB, (g + 1) * GB)
                    # --- stage 3: x2 = f + sign(f)*(u1*r1 + u2*r2) ---
                    xTb = grp.tile([P, 4, GB * P], BF16, tag="xT")
                    for mm in range(GB):
                        m = g * GB + mm
                        d1 = wk.tile([P, D_IN], F32, tag="d1")
                        nc.vector.tensor_scalar(out=d1[:],
                                                in0=u1g[g][:, mm, :],
                                                scalar1=r1a[:, m:m + 1],
                                                scalar2=None, op0=OP.mult)
                        dt = wk.tile([P, D_IN], F32, tag="dt")
                        nc.vector.scalar_tensor_tensor(
                            out=dt[:], in0=u2g[g][:, mm, :],
                            scalar=r2a[:, m:m + 1], in1=d1[:],
                            op0=OP.mult, op1=OP.add)
                        sd = wk.tile([P, D_IN], F32, tag="sd")
                        nc.gpsimd.tensor_tensor(out=sd[:], in0=dt[:],
                                                in1=sgnt[m][:], op=OP.mult)
                        x2 = wk.tile([P, D_IN], F32, tag="x2")
                        nc.gpsimd.tensor_tensor(out=x2[:], in0=sd[:],
                                                in1=ftg[g][:, mm, :],
                                                op=OP.add)
                        xps = psT.tile([P, 4, P], F32, tag="xps")
                        for c in range(4):
                            nc.tensor.transpose(xps[:, c, :],
                                                x2[:, c * P:(c + 1) * P],
                                                ident[:])
                        nc.scalar.activation(
                            xTb[:, :, mm * P:(mm + 1) * P], xps[:], AF.Copy)

                    # --- stage 4: projection for the group (free dim 512) ---
                    hps = psH.tile([P, GB * P], F32, tag="hp")
                    for c in range(4):
                        nc.tensor.matmul(hps[:], w1b[:, c, :], xTb[:, c, :],
                                         start=(c == 0), stop=(c == 3))
                    hT = grp.tile([P, GB * P], BF16, tag="hT")
                    nc.scalar.activation(hT[:], hps[:], AF.Relu, bias=b1t[:])
                    pps = psH.tile([P, GB * P], F32, tag="hp")
                    nc.tensor.matmul(pps[:], w2b[:], hT[:])
                    pT = grp.tile([P, GB * P], F32, tag="pT")
                    nc.scalar.activation(pT[:], pps[:], AF.Identity,
                                         bias=b2t[:])

                    # --- stage 5: p rows + per-row ||p||^2 ---
                    tppg = psQ.tile([P, GB, P], F32, tag="tppg")
                    for mm in range(GB):
                        m = g * GB + mm
                        nc.tensor.transpose(tppg[:, mm, :],
                                            pT[:, mm * P:(mm + 1) * P],
                                            ident[:])
                        njunk = sj.tile([P, P], BF16, tag="nj")
                        nc.scalar.activation(njunk[:], tppg[:, mm, :],
                                             AF.Square, bias=zbias[:],
                                             accum_out=nsqP[:, m:m + 1])

                    # --- stage 6: rsz = 1/||p|| with one Newton step; pos ---
                    n0 = small.tile([P, GB], F32, tag="n0")
                    nc.scalar.activation(n0[:], nsqP[:, g4], AF.Sqrt,
                                         bias=zbias[:])
                    rsz0 = small.tile([P, GB], F32, tag="rsz0")
                    nc.vector.reciprocal(rsz0[:], n0[:])
                    t1 = small.tile([P, GB], F32, tag="t1")
                    nc.vector.tensor_tensor(out=t1[:], in0=rsz0[:],
                                            in1=rsz0[:], op=OP.mult)
                    t2 = small.tile([P, GB], F32, tag="t2")
                    nc.vector.tensor_tensor(out=t2[:], in0=t1[:],
                                            in1=nsqP[:, g4], op=OP.mult)
                    t3 = small.tile([P, GB], F32, tag="t3")
                    nc.vector.tensor_scalar(out=t3[:], in0=t2[:], scalar1=-0.5,
                                            scalar2=1.5, op0=OP.mult,
                                            op1=OP.add)
                    rsz = small.tile([P, GB], F32, tag="rsz")
                    nc.vector.tensor_tensor(out=rsz[:], in0=rsz0[:],
                                            in1=t3[:], op=OP.mult)
                    av = small.tile([P, GB], F32, tag="av")
                    nc.vector.tensor_tensor(out=av[:], in0=nsqP[:, g4],
                                            in1=rsz[:], op=OP.mult)
                    # pos = nsq * rsz^2 / T  (diag of sim, fp32 path)
                    nc.vector.scalar_tensor_tensor(
                        out=outb[:, NBLK + g * GB:NBLK + (g + 1) * GB],
                        in0=av[:], scalar=INV_T, in1=rsz[:],
                        op0=OP.mult, op1=OP.mult)

                    # --- stage 7: z rows = p * rsz; transpose into zT bf16 ---
                    zrg = zr.tile([P, GB, P], BF16, tag="zrg")
                    ztpg = psZ.tile([P, GB, P], BF16, tag="ztpg")
                    for mm in range(GB):
                        nc.scalar.activation(zrg[:, mm, :], tppg[:, mm, :],
                                             AF.Copy, bias=0.0,
                                             scale=rsz[:, mm:mm + 1])
                        nc.tensor.transpose(ztpg[:, mm, :], zrg[:, mm, :],
                                            identb[:])
                    nc.vector.tensor_copy(zT[:, g4, :], ztpg[:])

                    # --- stage 8: ship this chunk of zT; AllGather it ---
                    nc.sync.dma_start(out=zTb[g][:, :, :], in_=zT[:, g4, :])
                    nc.gpsimd.collective_compute(
                        "AllGather",
                        OP.bypass,
                        ins=[zTb[g][:, :, :]],
                        outs=[zallb[g][:, :]],
                        replica_groups=[list(range(N_CORES))],
                    )

            # ---- land the gathered chunks in SBUF ----
            for g in range(NGRP):
                for r in range(N_CORES):
                    nc.sync.dma_start(out=zallT[g][:, r, :],
                                      in_=zallb[g][r * P:(r + 1) * P, :])

            # ======== Phase C: sim row-blocks + fused exp/rowsum =========
            with tc.tile_pool(name="psC", bufs=2, space="PSUM") as psC:
                for g in range(NGRP):
                    for m in range(NBLK):
                        lhsT = zT[:, m, :]
                        for h in range(2):
                            ps = psC.tile([P, 4 * 512], F32, tag="ps")
                            for j in range(4):
                                nc.tensor.matmul(
                                    ps[:, j * 512:(j + 1) * 512], lhsT,
                                    zallT[g][:, h * 4 + j, :])
                            eo = expsc.tile([P, 4 * 512], BF16, tag="eo",
                                            bufs=3)
                            k = 2 * g + h
                            if h == 0:
                                nc.scalar.activation(
                                    eo[:], ps[:], AF.Exp, bias=zbias[:],
                                    scale=INV_T,
                                    accum_out=sacc[:, m, k:k + 1])
                            else:
                                # row-sum on the (idle) vector engine to
                                # keep READ_ACCUMULATOR off the Act engine
                                nc.scalar.activation(
                                    eo[:], ps[:], AF.Exp, bias=zbias[:],
                                    scale=INV_T)
                                nc.vector.tensor_reduce(
                                    out=sacc[:, m, k:k + 1], in_=eo[:],
                                    axis=mybir.AxisListType.X, op=OP.add)

                # ---- final: logS per row; host does the scalar reduce ----
                for m in range(NBLK):
                    nc.vector.tensor_reduce(out=Stot[:, m:m + 1],
                                            in_=sacc[:, m, :],
                                            axis=mybir.AxisListType.X,
                                            op=OP.add)
                nc.scalar.activation(outb[:, 0:NBLK], Stot[:], AF.Ln,
                                     bias=zbias[:])
                nc.sync.dma_start(out=out_d[:, :], in_=outb[:])

    split_excess_waits(nc)
    return nc


_NC_CACHE = None


def _get_nc():
    global _NC_CACHE
    if _NC_CACHE is None:
        _NC_CACHE = build_nc()
    return _NC_CACHE


def finalize_outputs(core_outs) -> np.ndarray:
    """core_outs: list of per-core arrays 'out' [P, 2*NBLK] f32."""
    total = 0.0
    for arr in core_outs:
        a = np.asarray(arr, dtype=np.float64)
        total += a[:, :NBLK].sum() - a[:, NBLK:].sum()
    loss = total / float(N) + float(np.log(np.float32(2.0)))
    return np.array(loss, dtype=np.float32)


def run_spmd(inputs, trace=False, **kw):
    feats = np.ascontiguousarray(inputs["features"], dtype=np.float32)
    n1 = np.ascontiguousarray(inputs["noise1"], dtype=np.float32)
    n2 = np.ascontiguousarray(inputs["noise2"], dtype=np.float32)
    w1 = np.ascontiguousarray(inputs["W1"], dtype=np.float32)
    b1 = np.ascontiguousarray(inputs["b1"], dtype=np.float32).reshape(D_PROJ, 1)
    w2 = np.ascontiguousarray(inputs["W2"], dtype=np.float32)
    b2 = np.ascontiguousarray(inputs["b2"], dtype=np.float32).reshape(D_PROJ, 1)

    in_maps = []
    for r in range(N_CORES):
        sl = slice(r * ROWS, (r + 1) * ROWS)
        in_maps.append({
            "features": feats[sl], "noise1": n1[sl], "noise2": n2[sl],
            "W1": w1, "b1": b1, "W2": w2, "b2": b2,
        })
    nc = _get_nc()
    return run_bass_kernel_spmd(nc, in_maps, core_ids=list(range(N_CORES)),
                                trace=trace, **kw)


def kernel(**inputs) -> np.ndarray:
    out = run_spmd(inputs)
    return finalize_outputs([out.results[r]["out"] for r in range(N_CORES)])


# revision 32
# speedup vs baseline: 1.0311x; 1.0311x over previous
"""Distributed Trainium2 (Bass/Tile) kernel for the KPCL contrastive loss.

Math (matches the jax reference):
  x1 = f + sign(f) * normalize(n1, 1e-8) * 0.1
  x2 = x1 + sign(x1) * normalize(n2, 1e-8) * 0.1
     = sign(f) * (|f| + u1/max(10*||u1||,1e-7) + u2/max(10*||u2||,1e-7))
  p  = relu(x2 @ W1 + b1) @ W2 + b2
  z  = p / max(||p||, 1e-6)
  sim = z @ z_all.T / T ;  lse_i = log(sum_j exp(sim_ij)) ; pos_i = sim_ii
  loss = mean(-pos + lse) + log(2)

Sharding: rows (N=8192) split across 8 cores, 1024 rows each. Each core
computes its z block in transposed layout zT [128, 8, 128] (bf16), and
the zT columns are AllGathered in two 512-column chunks (each [128,512]
bf16, fired as soon as its half of the local rows is done so the
collective overlaps the rest of phase A). A tiny dummy AllGather is
issued first so the one-time collectives bootstrap barrier runs
concurrently with phase A instead of gating the real data transfers.
Phase C computes the row-block of sim = zT_m^T @ z_all^T as bf16
128x512 matmuls with fused exp+row-sum on the activation engine.
Per-core output is [128, 16] (per-partition log-sum-exp values and diag
terms); the host does the final scalar reduction.

Engine split in phase A: Act does squares/abs/sign/sqrt/relu, DVE does
the augment adds + norms, Pool (gpsimd) does the sign-multiply and all
PSUM->SBUF copies, PE does transposes + the (bf16) projection matmuls.
"""

import sys

for _p in ("/opt/trn_rl_repo",):
    if _p not in sys.path:
        sys.path.append(_p)

import numpy as np

import concourse.bass as bass
import concourse.tile as tile
from concourse import mybir
from concourse.bass_utils import run_bass_kernel_spmd
from concourse.masks import make_identity

F32 = mybir.dt.float32
BF16 = mybir.dt.bfloat16
U32 = mybir.dt.uint32

N_CORES = 8
N = 8192
ROWS = N // N_CORES          # 1024 rows per core
D_IN = 512
D_PROJ = 128
TEMP = 0.15
P = 128                      # partitions
NBLK = ROWS // P             # 8 row-blocks per core
GB = 4                       # blocks per group (AllGather chunk)
NGRP = NBLK // GB            # 2 groups
INV_T = 1.0 / TEMP

AF = mybir.ActivationFunctionType
OP = mybir.AluOpType


def split_excess_waits(nc: bass.Bass, max_waits: int = 1) -> int:
    """Hoist excess sem waits onto same-engine nop carriers.

    The walrus build in this image rejects instructions carrying more
    than ~2 sync commands ("Too many sync wait commands"), but Tile's
    wait assignment freely emits 2-3 waits per instruction. Splitting
    the waits onto preceding nop instructions on the same engine queue
    is semantically identical (engine program order is preserved).
    """
    nmoved = 0
    for f in nc.m.functions:
        for b in f.blocks:
            il = b.instructions
            i = 0
            while i < len(il):
                inst = il[i]
                si = inst.sync_info
                if si is None or not si.on_wait or len(si.on_wait) <= max_waits:
                    i += 1
                    continue
                eng = inst.engine
                if eng is None:
                    i += 1
                    continue
                waits = list(si.on_wait)
                keep = waits[-max_waits:]
                excess = waits[:-max_waits]
                carriers = []
                for w in excess:
                    nop = nc.engines[eng].nop().ins
                    for f2 in nc.m.functions:
                        for b2 in f2.blocks:
                            try:
                                b2.instructions.remove(nop)
                            except ValueError:
                                pass
                    nop.sync_info = mybir.SyncInfo(on_wait=[w], on_update=[])
                    carriers.append(nop)
                inst.sync_info = mybir.SyncInfo(on_wait=keep,
                                                on_update=list(si.on_update))
                for c in reversed(carriers):
                    il.insert(i, c)
                i += 1 + len(carriers)
                nmoved += len(excess)
    return nmoved


def build_nc() -> bass.Bass:
    nc = bass.Bass("TRN2", target_bir_lowering=False, debug=False,
                   num_devices=N_CORES)

    f_d = nc.dram_tensor("features", [ROWS, D_IN], F32, kind="ExternalInput")
    u1_d = nc.dram_tensor("noise1", [ROWS, D_IN], F32, kind="ExternalInput")
    u2_d = nc.dram_tensor("noise2", [ROWS, D_IN], F32, kind="ExternalInput")
    w1_d = nc.dram_tensor("W1", [D_IN, D_PROJ], F32, kind="ExternalInput")
    b1_d = nc.dram_tensor("b1", [D_PROJ, 1], F32, kind="ExternalInput")
    w2_d = nc.dram_tensor("W2", [D_PROJ, D_PROJ], F32, kind="ExternalInput")
    b2_d = nc.dram_tensor("b2", [D_PROJ, 1], F32, kind="ExternalInput")
    out_d = nc.dram_tensor("out", [P, 2 * NBLK], F32, kind="ExternalOutput")

    # dummy collective to pull the one-time bootstrap barrier early
    # (gathers a tiny scratch tensor whose garbage values are unused; it
    # has no producers so the trigger fires as soon as the CC core boots)
    dmy_in = nc.dram_tensor("dmy_in", [P, 1], F32)
    dmy_out = nc.dram_tensor("dmy_out", [N_CORES * P, 1], F32,
                             addr_space="Shared")

    # collective bounce buffers per chunk (AG output must be Shared)
    zTb = [nc.dram_tensor(f"zTb{g}", [P, GB, P], BF16) for g in range(NGRP)]
    zallb = [nc.dram_tensor(f"zallb{g}", [N_CORES * P, GB * P], BF16,
                            addr_space="Shared") for g in range(NGRP)]

    with tile.TileContext(nc) as tc:
        with (
            tc.tile_pool(name="singles", bufs=1) as singles,
            tc.tile_pool(name="grp", bufs=2) as grp,
            tc.tile_pool(name="wk", bufs=2) as wk,
            tc.tile_pool(name="sj", bufs=2) as sj,
            tc.tile_pool(name="zr", bufs=2) as zr,
            tc.tile_pool(name="small", bufs=2) as small,
            tc.tile_pool(name="expsc", bufs=2) as expsc,
        ):
            # ---- all input DMAs up front, spread across three engines'
            # DMA queues (each hardware DGE queue sustains only ~110 GB/s)
            ftg, u1g, u2g = [], [], []
            for g in range(NGRP):
                ftg.append(grp.tile([P, GB, D_IN], F32, name=f"ft{g}",
                                    tag="F"))
                u1g.append(grp.tile([P, GB, D_IN], F32, name=f"u1t{g}",
                                    tag="U1"))
                u2g.append(grp.tile([P, GB, D_IN], F32, name=f"u2t{g}",
                                    tag="U2"))
            for g in range(NGRP):
                for mm in range(GB):
                    rs = slice((g * GB + mm) * P, (g * GB + mm + 1) * P)
                    nc.sync.dma_start(u1g[g][:, mm, :], u1_d[rs, :])
                    nc.scalar.dma_start(ftg[g][:, mm, :], f_d[rs, :])
                    nc.gpsimd.dma_start(u2g[g][:, mm, :], u2_d[rs, :])

            # fire the dummy collective right after the cheap gpsimd DMA
            # issues: its completion is unused, it only exists to absorb
            # the one-time collectives bootstrap barrier early.
            nc.gpsimd.collective_compute(
                "AllGather", OP.bypass, ins=[dmy_in[:, :]],
                outs=[dmy_out[:, :]],
                replica_groups=[list(range(N_CORES))],
            )
            zbias = singles.tile([P, 1], F32)
            nc.gpsimd.memset(zbias[:], 0.0)

            # ---- constants / persistent tiles ----
            w1f = singles.tile([P, 4, P], F32)
            for c in range(4):
                nc.sync.dma_start(w1f[:, c, :], w1_d[c * P:(c + 1) * P, :])
            w2f = singles.tile([P, P], F32)
            nc.sync.dma_start(w2f[:], w2_d[:, :])
            b1t = singles.tile([P, 1], F32)
            nc.sync.dma_start(b1t[:], b1_d[:, :])
            b2t = singles.tile([P, 1], F32)
            nc.sync.dma_start(b2t[:], b2_d[:, :])
            w1b = singles.tile([P, 4, P], BF16)
            nc.vector.tensor_copy(w1b[:], w1f[:])
            w2b = singles.tile([P, P], BF16)
            nc.vector.tensor_copy(w2b[:], w2f[:])

            ident = singles.tile([P, P], F32)
            make_identity(nc, ident[:])
            identb = singles.tile([P, P], BF16)
            nc.vector.tensor_copy(identb[:], ident[:])

            s1all = singles.tile([P, NBLK], F32)    # ||u1||^2 per row
            s2all = singles.tile([P, NBLK], F32)
            nsqP = singles.tile([P, NBLK], F32)     # ||p||^2 per row (col layout)
            zT = singles.tile([P, NBLK, P], BF16)   # z^T for this core
            zallT = [singles.tile([P, N_CORES, GB * P], BF16,
                                  name=f"zallT{g}", tag=f"zallT{g}")
                     for g in range(NGRP)]
            sacc = singles.tile([P, NBLK, 4], F32)  # partial exp row-sums
            Stot = singles.tile([P, NBLK], F32)
            outb = singles.tile([P, 2 * NBLK], F32)  # [logS | pos]

            # =========== Phase A: augment + projection + normalize ==========
            with (
                tc.tile_pool(name="psT", bufs=2, space="PSUM") as psT,
                tc.tile_pool(name="psH", bufs=2, space="PSUM") as psH,
                tc.tile_pool(name="psQ", bufs=2, space="PSUM") as psQ,
                tc.tile_pool(name="psZ", bufs=2, space="PSUM") as psZ,
            ):
                # --- stage 1 (all blocks): row sums of squares, sign(f) ---
                sgnt = []
                for m in range(NBLK):
                    g, mm = divmod(m, GB)
                    junk = sj.tile([P, D_IN], BF16, tag="sqj")
                    nc.vector.scalar_tensor_tensor(
                        out=junk[:], in0=u1g[g][:, mm, :], scalar=1.0,
                        in1=u1g[g][:, mm, :], op0=OP.mult, op1=OP.mult,
                        accum_out=s1all[:, m:m + 1])
                    junk = sj.tile([P, D_IN], BF16, tag="sqj")
                    nc.vector.scalar_tensor_tensor(
                        out=junk[:], in0=u2g[g][:, mm, :], scalar=1.0,
                        in1=u2g[g][:, mm, :], op0=OP.mult, op1=OP.mult,
                        accum_out=s2all[:, m:m + 1])
                    sg = wk.tile([P, D_IN], F32, tag="sgn", bufs=9)
                    nc.scalar.activation(sg[:], ftg[g][:, mm, :], AF.Sign,
                                         bias=zbias[:])
                    sgnt.append(sg)

                # --- stage 2 (batched): r = 1/max(10*||u||, 1e-7) ---
                n1gt = small.tile([P, NBLK], F32, tag="n1g")
                nc.scalar.activation(n1gt[:], s1all[:], AF.Sqrt,
                                     bias=zbias[:], scale=100.0)
                n2gt = small.tile([P, NBLK], F32, tag="n2g")
                nc.scalar.activation(n2gt[:], s2all[:], AF.Sqrt,
                                     bias=zbias[:], scale=100.0)
                n1c = small.tile([P, NBLK], F32, tag="n1c")
                nc.vector.tensor_scalar(out=n1c[:], in0=n1gt[:],
                                        scalar1=1e-7, scalar2=None,
                                        op0=OP.max)
                r1a = small.tile([P, NBLK], F32, tag="r1a")
                nc.vector.reciprocal(r1a[:], n1c[:])
                n2c = small.tile([P, NBLK], F32, tag="n2c")
                nc.vector.tensor_scalar(out=n2c[:], in0=n2gt[:],
                                        scalar1=1e-7, scalar2=None,
                                        op0=OP.max)
                r2a = small.tile([P, NBLK], F32, tag="r2a")
                nc.vector.reciprocal(r2a[:], n2c[:])

                for g in range(NGRP):
                    g4 = slice(g * GB, (g + 1) * GB)
                    # --- stage 3: x2 = f + sign(f)*(u1*r1 + u2*r2) ---
                    xTb = grp.tile([P, 4, GB * P], BF16, tag="xT")
                    for mm in range(GB):
                        m = g * GB + mm
                        d1 = wk.tile([P, D_IN], F32, tag="d1")
                        nc.vector.tensor_scalar(out=d1[:],
                                                in0=u1g[g][:, mm, :],
                                                scalar1=r1a[:, m:m + 1],
                                                scalar2=None, op0=OP.mult)
                        dt = wk.tile([P, D_IN], F32, tag="dt")
                        nc.vector.scalar_tensor_tensor(
                            out=dt[:], in0=u2g[g][:, mm, :],
                            scalar=r2a[:, m:m + 1], in1=d1[:],
                            op0=OP.mult, op1=OP.add)
                        sd = wk.tile([P, D_IN], F32, tag="sd")
                        nc.gpsimd.tensor_tensor(out=sd[:], in0=dt[:],
                                                in1=sgnt[m][:], op=OP.mult)
                        x2 = wk.tile([P, D_IN], F32, tag="x2")
                        nc.gpsimd.tensor_tensor(out=x2[:], in0=sd[:],
                                                in1=ftg[g][:, mm, :],
                                                op=OP.add)
                        xps = psT.tile([P, 4, P], F32, tag="xps")
                        for c in range(4):
                            nc.tensor.transpose(xps[:, c, :],
                                                x2[:, c * P:(c + 1) * P],
                                                ident[:])
                        nc.scalar.activation(
                            xTb[:, :, mm * P:(mm + 1) * P], xps[:], AF.Copy)

                    # --- stage 4: projection for the group (free dim 512) ---
                    hps = psH.tile([P, GB * P], F32, tag="hp")
                    for c in range(4):
                        nc.tensor.matmul(hps[:], w1b[:, c, :], xTb[:, c, :],
                                         start=(c == 0), stop=(c == 3))
                    hT = grp.tile([P, GB * P], BF16, tag="hT")
                    nc.scalar.activation(hT[:], hps[:], AF.Relu, bias=b1t[:])
                    pps = psH.tile([P, GB * P], F32, tag="hp")
                    nc.tensor.matmul(pps[:], w2b[:], hT[:])
                    pT = grp.tile([P, GB * P], F32, tag="pT")
                    nc.scalar.activation(pT[:], pps[:], AF.Identity,
                                         bias=b2t[:])

                    # --- stage 5: p rows + per-row ||p||^2 ---
                    tppg = psQ.tile([P, GB, P], F32, tag="tppg")
                    for mm in range(GB):
                        m = g * GB + mm
                        nc.tensor.transpose(tppg[:, mm, :],
                                            pT[:, mm * P:(mm + 1) * P],
                                            ident[:])
                        njunk = sj.tile([P, P], BF16, tag="nj")
                        nc.scalar.activation(njunk[:], tppg[:, mm, :],
                                             AF.Square, bias=zbias[:],
                                             accum_out=nsqP[:, m:m + 1])

                    # --- stage 6: rsz = 1/||p|| with one Newton step; pos ---
                    n0 = small.tile([P, GB], F32, tag="n0")
                    nc.scalar.activation(n0[:], nsqP[:, g4], AF.Sqrt,
                                         bias=zbias[:])
                    rsz0 = small.tile([P, GB], F32, tag="rsz0")
                    nc.vector.reciprocal(rsz0[:], n0[:])
                    t1 = small.tile([P, GB], F32, tag="t1")
                    nc.vector.tensor_tensor(out=t1[:], in0=rsz0[:],
                                            in1=rsz0[:], op=OP.mult)
                    t2 = small.tile([P, GB], F32, tag="t2")
                    nc.vector.tensor_tensor(out=t2[:], in0=t1[:],
                                            in1=nsqP[:, g4], op=OP.mult)
                    t3 = small.tile([P, GB], F32, tag="t3")
                    nc.vector.tensor_scalar(out=t3[:], in0=t2[:], scalar1=-0.5,
                                            scalar2=1.5, op0=OP.mult,
                                            op1=OP.add)
                    rsz = small.tile([P, GB], F32, tag="rsz")
                    nc.vector.tensor_tensor(out=rsz[:], in0=rsz0[:],
                                            in1=t3[:], op=OP.mult)
                    av = small.tile([P, GB], F32, tag="av")
                    nc.vector.tensor_tensor(out=av[:], in0=nsqP[:, g4],
                                            in1=rsz[:], op=OP.mult)
                    # pos = nsq * rsz^2 / T  (diag of sim, fp32 path)
                    nc.vector.scalar_tensor_tensor(
                        out=outb[:, NBLK + g * GB:NBLK + (g + 1) * GB],
                        in0=av[:], scalar=INV_T, in1=rsz[:],
                        op0=OP.mult, op1=OP.mult)

                    # --- stage 7: z rows = p * rsz; transpose into zT bf16 ---
                    zrg = zr.tile([P, GB, P], BF16, tag="zrg")
                    ztpg = psZ.tile([P, GB, P], BF16, tag="ztpg")
                    for mm in range(GB):
                        nc.scalar.activation(zrg[:, mm, :], tppg[:, mm, :],
                                             AF.Copy, bias=0.0,
                                             scale=rsz[:, mm:mm + 1])
                        nc.tensor.transpose(ztpg[:, mm, :], zrg[:, mm, :],
                                            identb[:])
                    nc.vector.tensor_copy(zT[:, g4, :], ztpg[:])

                    # --- stage 8: ship this chunk of zT; AllGather it ---
                    nc.sync.dma_start(out=zTb[g][:, :, :], in_=zT[:, g4, :])
                    nc.gpsimd.collective_compute(
                        "AllGather",
                        OP.bypass,
                        ins=[zTb[g][:, :, :]],
                        outs=[zallb[g][:, :]],
                        replica_groups=[list(range(N_CORES))],
                    )

            # ---- land the gathered chunks in SBUF (Pool DGE queue: its
            # issue cost is tiny and the queue is idle during phase C) ----
            for g in range(NGRP):
                for r in range(N_CORES):
                    nc.gpsimd.dma_start(out=zallT[g][:, r, :],
                                        in_=zallb[g][r * P:(r + 1) * P, :])

            # ======== Phase C: sim row-blocks + fused exp/rowsum =========
            with tc.tile_pool(name="psC", bufs=2, space="PSUM") as psC:
                for g in range(NGRP):
                    for m in range(NBLK):
                        lhsT = zT[:, m, :]
                        for h in range(2):
                            ps = psC.tile([P, 4 * 512], F32, tag="ps")
                            for j in range(4):
                                nc.tensor.matmul(
                                    ps[:, j * 512:(j + 1) * 512], lhsT,
                                    zallT[g][:, h * 4 + j, :])
                            eo = expsc.tile([P, 4 * 512], BF16, tag="eo",
                                            bufs=3)
                            k = 2 * g + h
                            if h == 0:
                                nc.scalar.activation(
                                    eo[:], ps[:], AF.Exp, bias=zbias[:],
                                    scale=INV_T,
                                    accum_out=sacc[:, m, k:k + 1])
                            else:
                                # row-sum on the (idle) vector engine to
                                # keep READ_ACCUMULATOR off the Act engine
                                nc.scalar.activation(
                                    eo[:], ps[:], AF.Exp, bias=zbias[:],
                                    scale=INV_T)
                                nc.vector.tensor_reduce(
                                    out=sacc[:, m, k:k + 1], in_=eo[:],
                                    axis=mybir.AxisListType.X, op=OP.add)

                # ---- final: logS per row; host does the scalar reduce ----
                for m in range(NBLK):
                    nc.vector.tensor_reduce(out=Stot[:, m:m + 1],
                                            in_=sacc[:, m, :],
                                            axis=mybir.AxisListType.X,
                                            op=OP.add)
                nc.scalar.activation(outb[:, 0:NBLK], Stot[:], AF.Ln,
                                     bias=zbias[:])
                nc.sync.dma_start(out=out_d[:, :], in_=outb[:])

    split_excess_waits(nc)
    return nc


_NC_CACHE = None


def _get_nc():
    global _NC_CACHE
    if _NC_CACHE is None:
        _NC_CACHE = build_nc()
    return _NC_CACHE


def finalize_outputs(core_outs) -> np.ndarray:
    """core_outs: list of per-core arrays 'out' [P, 2*NBLK] f32."""
    total = 0.0
    for arr in core_outs:
        a = np.asarray(arr, dtype=np.float64)
        total += a[:, :NBLK].sum() - a[:, NBLK:].sum()
    loss = total / float(N) + float(np.log(np.float32(2.0)))
    return np.array(loss, dtype=np.float32)


def run_spmd(inputs, trace=False, **kw):
    feats = np.ascontiguousarray(inputs["features"], dtype=np.float32)
    n1 = np.ascontiguousarray(inputs["noise1"], dtype=np.float32)
    n2 = np.ascontiguousarray(inputs["noise2"], dtype=np.float32)
    w1 = np.ascontiguousarray(inputs["W1"], dtype=np.float32)
    b1 = np.ascontiguousarray(inputs["b1"], dtype=np.float32).reshape(D_PROJ, 1)
    w2 = np.ascontiguousarray(inputs["W2"], dtype=np.float32)
    b2 = np.ascontiguousarray(inputs["b2"], dtype=np.float32).reshape(D_PROJ, 1)

    in_maps = []
    for r in range(N_CORES):
        sl = slice(r * ROWS, (r + 1) * ROWS)
        in_maps.append({
            "features": feats[sl], "noise1": n1[sl], "noise2": n2[sl],
            "W1": w1, "b1": b1, "W2": w2, "b2": b2,
        })
    nc = _get_nc()
    return run_bass_kernel_spmd(nc, in_maps, core_ids=list(range(N_CORES)),
                                trace=trace, **kw)


def kernel(**inputs) -> np.ndarray:
    out = run_spmd(inputs)
    return finalize_outputs([out.results[r]["out"] for r in range(N_CORES)])


# revision 33
# speedup vs baseline: 1.2167x; 1.1800x over previous
"""Distributed Trainium2 (Bass/Tile) kernel for the KPCL contrastive loss.

Math (matches the jax reference):
  x1 = f + sign(f) * normalize(n1, 1e-8) * 0.1
  x2 = x1 + sign(x1) * normalize(n2, 1e-8) * 0.1
     = sign(f) * (|f| + u1/max(10*||u1||,1e-7) + u2/max(10*||u2||,1e-7))
  p  = relu(x2 @ W1 + b1) @ W2 + b2
  z  = p / max(||p||, 1e-6)
  sim = z @ z_all.T / T ;  lse_i = log(sum_j exp(sim_ij)) ; pos_i = sim_ii
  loss = mean(-pos + lse) + log(2)

Sharding: rows (N=8192) split across 8 cores, 1024 rows each. Each core
computes its z block in transposed layout zT [128, 8, 128] (bf16). The
zT columns are AllGathered in four 256-column chunks, each fired as soon
as its 2 row-blocks finish so the collectives overlap the rest of
phase A and each other's exposure to wire-time variance is small. A
tiny dummy AllGather with no producers is issued first so the one-time
collectives bootstrap barrier (CC core boots ~20us in, rendezvous
~30us) runs concurrently with phase A instead of gating real data.
Phase C computes the row-blocks of sim = zT_m^T @ z_all^T as bf16
128x512 matmuls with fused exp+row-sum on the activation engine (the
hard floor: 8M exps/core at ~0.9ns each).

Engine notes: input loads are spread over the SP and Act DMA queues
(a single hardware DGE queue sustains only ~110 GB/s); Pool (gpsimd)
does the two sign-multiply passes (it cannot touch PSUM); PSUM->SBUF
copies ride Act/DVE; half the phase C row sums use the Act
accumulator, half use DVE reduces, to split the overhead.
"""

import sys

for _p in ("/opt/trn_rl_repo",):
    if _p not in sys.path:
        sys.path.append(_p)

import numpy as np

import concourse.bass as bass
import concourse.tile as tile
from concourse import mybir
from concourse.bass_utils import run_bass_kernel_spmd
from concourse.masks import make_identity

F32 = mybir.dt.float32
BF16 = mybir.dt.bfloat16
U32 = mybir.dt.uint32

N_CORES = 8
N = 8192
ROWS = N // N_CORES          # 1024 rows per core
D_IN = 512
D_PROJ = 128
TEMP = 0.15
P = 128                      # partitions
NBLK = ROWS // P             # 8 row-blocks per core
GB = 2                       # blocks per group (AllGather chunk)
NGRP = NBLK // GB            # 4 groups
INV_T = 1.0 / TEMP

AF = mybir.ActivationFunctionType
OP = mybir.AluOpType


def split_excess_waits(nc: bass.Bass, max_waits: int = 1) -> int:
    """Hoist excess sem waits onto same-engine nop carriers.

    The walrus build in this image rejects instructions carrying more
    than ~2 sync commands ("Too many sync wait commands"), but Tile's
    wait assignment freely emits 2-3 waits per instruction. Splitting
    the waits onto preceding nop instructions on the same engine queue
    is semantically identical (engine program order is preserved).
    """
    nmoved = 0
    for f in nc.m.functions:
        for b in f.blocks:
            il = b.instructions
            i = 0
            while i < len(il):
                inst = il[i]
                si = inst.sync_info
                if si is None or not si.on_wait or len(si.on_wait) <= max_waits:
                    i += 1
                    continue
                eng = inst.engine
                if eng is None:
                    i += 1
                    continue
                waits = list(si.on_wait)
                keep = waits[-max_waits:]
                excess = waits[:-max_waits]
                carriers = []
                for w in excess:
                    nop = nc.engines[eng].nop().ins
                    for f2 in nc.m.functions:
                        for b2 in f2.blocks:
                            try:
                                b2.instructions.remove(nop)
                            except ValueError:
                                pass
                    nop.sync_info = mybir.SyncInfo(on_wait=[w], on_update=[])
                    carriers.append(nop)
                inst.sync_info = mybir.SyncInfo(on_wait=keep,
                                                on_update=list(si.on_update))
                for c in reversed(carriers):
                    il.insert(i, c)
                i += 1 + len(carriers)
                nmoved += len(excess)
    return nmoved


def build_nc() -> bass.Bass:
    nc = bass.Bass("TRN2", target_bir_lowering=False, debug=False,
                   num_devices=N_CORES)

    f_d = nc.dram_tensor("features", [ROWS, D_IN], F32, kind="ExternalInput")
    u1_d = nc.dram_tensor("noise1", [ROWS, D_IN], F32, kind="ExternalInput")
    u2_d = nc.dram_tensor("noise2", [ROWS, D_IN], F32, kind="ExternalInput")
    w1_d = nc.dram_tensor("W1", [D_IN, D_PROJ], F32, kind="ExternalInput")
    b1_d = nc.dram_tensor("b1", [D_PROJ, 1], F32, kind="ExternalInput")
    w2_d = nc.dram_tensor("W2", [D_PROJ, D_PROJ], F32, kind="ExternalInput")
    b2_d = nc.dram_tensor("b2", [D_PROJ, 1], F32, kind="ExternalInput")
    out_d = nc.dram_tensor("out", [P, 2 * NBLK], F32, kind="ExternalOutput")

    # dummy collective to pull the one-time bootstrap barrier early
    # (gathers a tiny scratch tensor whose garbage values are unused; it
    # has no producers so the trigger fires as soon as the CC core boots)
    dmy_in = nc.dram_tensor("dmy_in", [P, 1], F32)
    dmy_out = nc.dram_tensor("dmy_out", [N_CORES * P, 1], F32,
                             addr_space="Shared")

    # collective bounce buffers per chunk (AG output must be Shared)
    zTb = [nc.dram_tensor(f"zTb{g}", [P, GB, P], BF16) for g in range(NGRP)]
    zallb = [nc.dram_tensor(f"zallb{g}", [N_CORES * P, GB * P], BF16,
                            addr_space="Shared") for g in range(NGRP)]

    with tile.TileContext(nc) as tc:
        with (
            tc.tile_pool(name="singles", bufs=1) as singles,
            tc.tile_pool(name="grp", bufs=3) as grp,
            tc.tile_pool(name="wk", bufs=2) as wk,
            tc.tile_pool(name="sj", bufs=2) as sj,
            tc.tile_pool(name="zr", bufs=2) as zr,
            tc.tile_pool(name="small", bufs=2) as small,
            tc.tile_pool(name="expsc", bufs=2) as expsc,
        ):
            # fire the dummy collective as the first gpsimd instruction:
            # its completion is unused, it only absorbs the bootstrap
            # barrier while phase A runs.
            nc.gpsimd.collective_compute(
                "AllGather", OP.bypass, ins=[dmy_in[:, :]],
                outs=[dmy_out[:, :]],
                replica_groups=[list(range(N_CORES))],
            )
            zbias = singles.tile([P, 1], F32)
            nc.gpsimd.memset(zbias[:], 0.0)

            # ---- all input DMAs up front; spread u1/u2 (SP) and f (Act)
            # across two hardware DGE queues (~110 GB/s each) ----
            ftg, u1g, u2g = [], [], []
            for g in range(NGRP):
                ftg.append(grp.tile([P, GB, D_IN], F32, name=f"ft{g}",
                                    tag="F"))
                u1g.append(grp.tile([P, GB, D_IN], F32, name=f"u1t{g}",
                                    tag="U1"))
                u2g.append(grp.tile([P, GB, D_IN], F32, name=f"u2t{g}",
                                    tag="U2"))
            for g in range(NGRP):
                for mm in range(GB):
                    rs = slice((g * GB + mm) * P, (g * GB + mm + 1) * P)
                    nc.sync.dma_start(u1g[g][:, mm, :], u1_d[rs, :])
                    nc.sync.dma_start(u2g[g][:, mm, :], u2_d[rs, :])
                    nc.scalar.dma_start(ftg[g][:, mm, :], f_d[rs, :])

            # ---- constants / persistent tiles ----
            w1f = singles.tile([P, 4, P], F32)
            for c in range(4):
                nc.sync.dma_start(w1f[:, c, :], w1_d[c * P:(c + 1) * P, :])
            w2f = singles.tile([P, P], F32)
            nc.sync.dma_start(w2f[:], w2_d[:, :])
            b1t = singles.tile([P, 1], F32)
            nc.sync.dma_start(b1t[:], b1_d[:, :])
            b2t = singles.tile([P, 1], F32)
            nc.sync.dma_start(b2t[:], b2_d[:, :])
            w1b = singles.tile([P, 4, P], BF16)
            nc.vector.tensor_copy(w1b[:], w1f[:])
            w2b = singles.tile([P, P], BF16)
            nc.vector.tensor_copy(w2b[:], w2f[:])

            ident = singles.tile([P, P], F32)
            make_identity(nc, ident[:])
            identb = singles.tile([P, P], BF16)
            nc.vector.tensor_copy(identb[:], ident[:])

            s1all = singles.tile([P, NBLK], F32)    # ||u1||^2 per row
            s2all = singles.tile([P, NBLK], F32)
            nsqP = singles.tile([P, NBLK], F32)     # ||p||^2 per row
            zT = singles.tile([P, NBLK, P], BF16)   # z^T for this core
            zallT = [singles.tile([P, N_CORES, GB * P], BF16,
                                  name=f"zallT{g}", tag=f"zallT{g}")
                     for g in range(NGRP)]
            sacc = singles.tile([P, NBLK, NGRP], F32)  # partial exp row-sums
            Stot = singles.tile([P, NBLK], F32)
            outb = singles.tile([P, 2 * NBLK], F32)  # [logS | pos]

            # =========== Phase A: augment + projection + normalize ==========
            with (
                tc.tile_pool(name="psT", bufs=2, space="PSUM") as psT,
                tc.tile_pool(name="psH", bufs=2, space="PSUM") as psH,
                tc.tile_pool(name="psQ", bufs=2, space="PSUM") as psQ,
                tc.tile_pool(name="psZ", bufs=2, space="PSUM") as psZ,
            ):
                for g in range(NGRP):
                    gsl = slice(g * GB, (g + 1) * GB)
                    # --- stage 1: row sums of squares, sign(f) ---
                    sgnt = []
                    for mm in range(GB):
                        m = g * GB + mm
                        junk = sj.tile([P, D_IN], BF16, tag="sqj")
                        nc.vector.scalar_tensor_tensor(
                            out=junk[:], in0=u1g[g][:, mm, :], scalar=1.0,
                            in1=u1g[g][:, mm, :], op0=OP.mult, op1=OP.mult,
                            accum_out=s1all[:, m:m + 1])
                        junk = sj.tile([P, D_IN], BF16, tag="sqj")
                        nc.vector.scalar_tensor_tensor(
                            out=junk[:], in0=u2g[g][:, mm, :], scalar=1.0,
                            in1=u2g[g][:, mm, :], op0=OP.mult, op1=OP.mult,
                            accum_out=s2all[:, m:m + 1])
                        sg = wk.tile([P, D_IN], F32, tag="sgn", bufs=3)
                        nc.scalar.activation(sg[:], ftg[g][:, mm, :],
                                             AF.Sign, bias=zbias[:])
                        sgnt.append(sg)

                    # --- stage 2: noise scales r = 1/max(10*||u||,1e-7) ---
                    n1gt = small.tile([P, GB], F32, tag="n1g")
                    nc.scalar.activation(n1gt[:], s1all[:, gsl], AF.Sqrt,
                                         bias=zbias[:], scale=100.0)
                    n2gt = small.tile([P, GB], F32, tag="n2g")
                    nc.scalar.activation(n2gt[:], s2all[:, gsl], AF.Sqrt,
                                         bias=zbias[:], scale=100.0)
                    n1c = small.tile([P, GB], F32, tag="n1c")
                    nc.vector.tensor_scalar(out=n1c[:], in0=n1gt[:],
                                            scalar1=1e-7, scalar2=None,
                                            op0=OP.max)
                    r1g = small.tile([P, GB], F32, tag="r1g")
                    nc.vector.reciprocal(r1g[:], n1c[:])
                    n2c = small.tile([P, GB], F32, tag="n2c")
                    nc.vector.tensor_scalar(out=n2c[:], in0=n2gt[:],
                                            scalar1=1e-7, scalar2=None,
                                            op0=OP.max)
                    r2g = small.tile([P, GB], F32, tag="r2g")
                    nc.vector.reciprocal(r2g[:], n2c[:])

                    # --- stage 3: x2 = f + sign(f)*(u1*r1 + u2*r2) ---
                    xTb = grp.tile([P, 4, GB * P], BF16, tag="xT")
                    for mm in range(GB):
                        m = g * GB + mm
                        d1 = wk.tile([P, D_IN], F32, tag="d1")
                        nc.vector.tensor_scalar(out=d1[:],
                                                in0=u1g[g][:, mm, :],
                                                scalar1=r1g[:, mm:mm + 1],
                                                scalar2=None, op0=OP.mult)
                        dt = wk.tile([P, D_IN], F32, tag="dt")
                        nc.vector.scalar_tensor_tensor(
                            out=dt[:], in0=u2g[g][:, mm, :],
                            scalar=r2g[:, mm:mm + 1], in1=d1[:],
                            op0=OP.mult, op1=OP.add)
                        sd = wk.tile([P, D_IN], F32, tag="sd")
                        nc.gpsimd.tensor_tensor(out=sd[:], in0=dt[:],
                                                in1=sgnt[mm][:], op=OP.mult)
                        x2 = wk.tile([P, D_IN], F32, tag="x2")
                        nc.gpsimd.tensor_tensor(out=x2[:], in0=sd[:],
                                                in1=ftg[g][:, mm, :],
                                                op=OP.add)
                        xps = psT.tile([P, 4, P], F32, tag="xps")
                        for c in range(4):
                            nc.tensor.transpose(xps[:, c, :],
                                                x2[:, c * P:(c + 1) * P],
                                                ident[:])
                        nc.scalar.activation(
                            xTb[:, :, mm * P:(mm + 1) * P], xps[:], AF.Copy)

                    # --- stage 4: projection for the group ---
                    hps = psH.tile([P, GB * P], F32, tag="hp")
                    for c in range(4):
                        nc.tensor.matmul(hps[:], w1b[:, c, :], xTb[:, c, :],
                                         start=(c == 0), stop=(c == 3))
                    hT = grp.tile([P, GB * P], BF16, tag="hT")
                    nc.scalar.activation(hT[:], hps[:], AF.Relu, bias=b1t[:])
                    pps = psH.tile([P, GB * P], F32, tag="hp")
                    nc.tensor.matmul(pps[:], w2b[:], hT[:])
                    pT = grp.tile([P, GB * P], F32, tag="pT")
                    nc.scalar.activation(pT[:], pps[:], AF.Identity,
                                         bias=b2t[:])

                    # --- stage 5: p rows + per-row ||p||^2 ---
                    tppg = psQ.tile([P, GB, P], F32, tag="tppg")
                    for mm in range(GB):
                        m = g * GB + mm
                        nc.tensor.transpose(tppg[:, mm, :],
                                            pT[:, mm * P:(mm + 1) * P],
                                            ident[:])
                        njunk = sj.tile([P, P], BF16, tag="nj")
                        nc.scalar.activation(njunk[:], tppg[:, mm, :],
                                             AF.Square, bias=zbias[:],
                                             accum_out=nsqP[:, m:m + 1])

                    # --- stage 6: rsz = 1/||p|| with one Newton step; pos ---
                    n0 = small.tile([P, GB], F32, tag="n0")
                    nc.scalar.activation(n0[:], nsqP[:, gsl], AF.Sqrt,
                                         bias=zbias[:])
                    rsz0 = small.tile([P, GB], F32, tag="rsz0")
                    nc.vector.reciprocal(rsz0[:], n0[:])
                    t1 = small.tile([P, GB], F32, tag="t1")
                    nc.vector.tensor_tensor(out=t1[:], in0=rsz0[:],
                                            in1=rsz0[:], op=OP.mult)
                    t2 = small.tile([P, GB], F32, tag="t2")
                    nc.vector.tensor_tensor(out=t2[:], in0=t1[:],
                                            in1=nsqP[:, gsl], op=OP.mult)
                    t3 = small.tile([P, GB], F32, tag="t3")
                    nc.vector.tensor_scalar(out=t3[:], in0=t2[:],
                                            scalar1=-0.5, scalar2=1.5,
                                            op0=OP.mult, op1=OP.add)
                    rsz = small.tile([P, GB], F32, tag="rsz")
                    nc.vector.tensor_tensor(out=rsz[:], in0=rsz0[:],
                                            in1=t3[:], op=OP.mult)
                    av = small.tile([P, GB], F32, tag="av")
                    nc.vector.tensor_tensor(out=av[:], in0=nsqP[:, gsl],
                                            in1=rsz[:], op=OP.mult)
                    # pos = nsq * rsz^2 / T  (diag of sim, fp32 path)
                    nc.vector.scalar_tensor_tensor(
                        out=outb[:, NBLK + g * GB:NBLK + (g + 1) * GB],
                        in0=av[:], scalar=INV_T, in1=rsz[:],
                        op0=OP.mult, op1=OP.mult)

                    # --- stage 7: z rows = p * rsz; transpose into zT ---
                    zrg = zr.tile([P, GB, P], BF16, tag="zrg")
                    ztpg = psZ.tile([P, GB, P], BF16, tag="ztpg")
                    for mm in range(GB):
                        nc.scalar.activation(zrg[:, mm, :], tppg[:, mm, :],
                                             AF.Copy, bias=0.0,
                                             scale=rsz[:, mm:mm + 1])
                        nc.tensor.transpose(ztpg[:, mm, :], zrg[:, mm, :],
                                            identb[:])
                    nc.vector.tensor_copy(zT[:, gsl, :], ztpg[:])

                    # --- stage 8: ship this chunk of zT; AllGather it ---
                    nc.sync.dma_start(out=zTb[g][:, :, :], in_=zT[:, gsl, :])
                    nc.gpsimd.collective_compute(
                        "AllGather",
                        OP.bypass,
                        ins=[zTb[g][:, :, :]],
                        outs=[zallb[g][:, :]],
                        replica_groups=[list(range(N_CORES))],
                    )

            # ---- land the gathered chunks in SBUF ----
            for g in range(NGRP):
                for r in range(N_CORES):
                    nc.sync.dma_start(out=zallT[g][:, r, :],
                                      in_=zallb[g][r * P:(r + 1) * P, :])

            # ======== Phase C: sim row-blocks + fused exp/rowsum =========
            with tc.tile_pool(name="psC", bufs=2, space="PSUM") as psC:
                for g in range(NGRP):
                    for m in range(NBLK):
                        lhsT = zT[:, m, :]
                        ps = psC.tile([P, 4 * 512], F32, tag="ps")
                        for j in range(4):
                            nc.tensor.matmul(
                                ps[:, j * 512:(j + 1) * 512], lhsT,
                                zallT[g][:, 2 * j:2 * j + 2, :])
                        eo = expsc.tile([P, 4 * 512], BF16, tag="eo",
                                        bufs=3)
                        if g % 2 == 0:
                            nc.scalar.activation(
                                eo[:], ps[:], AF.Exp, bias=zbias[:],
                                scale=INV_T,
                                accum_out=sacc[:, m, g:g + 1])
                        else:
                            # row-sum on the (idle) vector engine to keep
                            # READ_ACCUMULATOR off the Act engine
                            nc.scalar.activation(
                                eo[:], ps[:], AF.Exp, bias=zbias[:],
                                scale=INV_T)
                            nc.vector.tensor_reduce(
                                out=sacc[:, m, g:g + 1], in_=eo[:],
                                axis=mybir.AxisListType.X, op=OP.add)

                # ---- final: logS per row; host does the scalar reduce ----
                for m in range(NBLK):
                    nc.vector.tensor_reduce(out=Stot[:, m:m + 1],
                                            in_=sacc[:, m, :],
                                            axis=mybir.AxisListType.X,
                                            op=OP.add)
                nc.scalar.activation(outb[:, 0:NBLK], Stot[:], AF.Ln,
                                     bias=zbias[:])
                nc.sync.dma_start(out=out_d[:, :], in_=outb[:])

    split_excess_waits(nc)
    return nc


_NC_CACHE = None


def _get_nc():
    global _NC_CACHE
    if _NC_CACHE is None:
        _NC_CACHE = build_nc()
    return _NC_CACHE


def finalize_outputs(core_outs) -> np.ndarray:
    """core_outs: list of per-core arrays 'out' [P, 2*NBLK] f32."""
    total = 0.0
    for arr in core_outs:
        a = np.asarray(arr, dtype=np.float64)
        total += a[:, :NBLK].sum() - a[:, NBLK:].sum()
    loss = total / float(N) + float(np.log(np.float32(2.0)))
    return np.array(loss, dtype=np.float32)


def run_spmd(inputs, trace=False, **kw):
    feats = np.ascontiguousarray(inputs["features"], dtype=np.float32)
    n1 = np.ascontiguousarray(inputs["noise1"], dtype=np.float32)
    n2 = np.ascontiguousarray(inputs["noise2"], dtype=np.float32)
    w1 = np.ascontiguousarray(inputs["W1"], dtype=np.float32)
    b1 = np.ascontiguousarray(inputs["b1"], dtype=np.float32).reshape(D_PROJ, 1)
    w2 = np.ascontiguousarray(inputs["W2"], dtype=np.float32)
    b2 = np.ascontiguousarray(inputs["b2"], dtype=np.float32).reshape(D_PROJ, 1)

    in_maps = []
    for r in range(N_CORES):
        sl = slice(r * ROWS, (r + 1) * ROWS)
        in_maps.append({
            "features": feats[sl], "noise1": n1[sl], "noise2": n2[sl],
            "W1": w1, "b1": b1, "W2": w2, "b2": b2,
        })
    nc = _get_nc()
    return run_bass_kernel_spmd(nc, in_maps, core_ids=list(range(N_CORES)),
                                trace=trace, **kw)


def kernel(**inputs) -> np.ndarray:
    out = run_spmd(inputs)
    return finalize_outputs([out.results[r]["out"] for r in range(N_CORES)])
